# revision 5
# baseline (speedup 1.0000x reference)
"""Trainium2 Bass kernel for nn_Codec (exponential-lr SGD codec rollout).

Math: the reference scan is affine in x. With lr_t = LR0*GAMMA**t and
c_0 = 0, c_{t+1} = (1-lr_t)*c_t + lr_t, the per-step outputs are
  spike_t = 0.5*(c_t - 1) * x + 0.5
  y_t     = c_{t+1} * x
so each of the 2*T output slices is a scalar affine map of x. The kernel
is therefore pure output-bandwidth: load the x shard once per core, emit
2*T scaled copies.

Sharding: rows of x split evenly across 8 cores (fully data parallel).
"""

import sys

import numpy as np

sys.path.insert(0, "/opt/trn_rl_repo")

import concourse.bass as bass
import concourse.bacc as bacc
import concourse.mybir as mybir
from concourse import tile
from concourse.bass_utils import run_bass_kernel_spmd

LR0 = 0.15
GAMMA = 0.95
N_CORES = 8
ROWS, COLS = 2048, 2048
SHARD = ROWS // N_CORES  # 256 rows per core
P = 128  # SBUF partitions

last_exec_time_ns = None

_nc_cache: dict[int, bass.Bass] = {}


def _coeffs(T: int) -> tuple[np.ndarray, np.ndarray]:
    lrs = LR0 * GAMMA ** np.arange(T, dtype=np.float64)
    c = np.zeros(T + 1)
    for t in range(T):
        c[t + 1] = (1.0 - lrs[t]) * c[t] + lrs[t]
    a_spike = (0.5 * (c[:T] - 1.0)).astype(np.float32)  # spike_t = a*x + 0.5
    a_y = c[1:].astype(np.float32)  # y_t = a*x
    return a_spike, a_y


def _build(T: int) -> bass.Bass:
    a_spike, a_y = _coeffs(T)
    f32 = mybir.dt.float32

    nc = bacc.Bacc("TRN2", target_bir_lowering=False)
    x = nc.dram_tensor("x", [SHARD, COLS], f32, kind="ExternalInput")
    out = nc.dram_tensor("out", [2, T, SHARD, COLS], f32, kind="ExternalOutput")

    n_row_tiles = SHARD // P  # 2
    with tile.TileContext(nc) as tc:
        with (
            tc.tile_pool(name="xin", bufs=1) as xpool,
            tc.tile_pool(name="obuf", bufs=8) as opool,
        ):
            xts = []
            for i in range(n_row_tiles):
                xt = xpool.tile([P, COLS], f32, tag=f"x{i}")
                nc.sync.dma_start(xt[:], x[i * P : (i + 1) * P, :])
                xts.append(xt)

            k = 0
            for t in range(T):
                for s, a, b in ((0, a_spike[t], 0.5), (1, a_y[t], 0.0)):
                    for i in range(n_row_tiles):
                        ot = opool.tile([P, COLS], f32, tag="o")
                        if k % 2 == 0:
                            nc.vector.tensor_scalar(
                                ot[:], xts[i][:], float(a), float(b),
                                mybir.AluOpType.mult, mybir.AluOpType.add,
                            )
                        else:
                            nc.scalar.activation(
                                ot[:], xts[i][:],
                                mybir.ActivationFunctionType.Copy,
                                bias=float(b), scale=float(a),
                            )
                        nc.sync.dma_start(out[s, t, i * P : (i + 1) * P, :], ot[:])
                        k += 1
    nc.finalize()
    return nc


def kernel(x: np.ndarray, T) -> np.ndarray:
    T = int(T)
    x = np.ascontiguousarray(np.asarray(x), dtype=np.float32)

    if T not in _nc_cache:
        _nc_cache[T] = _build(T)
    nc = _nc_cache[T]

    in_maps = [{"x": x[i * SHARD : (i + 1) * SHARD]} for i in range(N_CORES)]
    res = run_bass_kernel_spmd(nc, in_maps, list(range(N_CORES)))
    return np.concatenate([r["out"] for r in res.results], axis=2)


# revision 6
# speedup vs baseline: 265.9075x; 265.9075x over previous
"""Trainium2 Bass kernel for nn_Codec (exponential-lr SGD codec rollout).

Math: the reference scan is affine in x. With lr_t = LR0*GAMMA**t and
c_0 = 0, c_{t+1} = (1-lr_t)*c_t + lr_t, the per-step outputs are
  spike_t = 0.5*(c_t - 1) * x + 0.5
  y_t     = c_{t+1} * x
so each of the 2*T output slices is a scalar affine map of x. The kernel
is therefore pure output-bandwidth: load the x shard once per core, emit
2*T scaled copies.

Sharding: rows of x split evenly across 8 cores (fully data parallel).
"""

import sys

import numpy as np

sys.path.insert(0, "/opt/trn_rl_repo")

import concourse.bass as bass
import concourse.bacc as bacc
import concourse.mybir as mybir
from concourse import tile
from concourse.bass_utils import run_bass_kernel_spmd

LR0 = 0.15
GAMMA = 0.95
N_CORES = 8
ROWS, COLS = 2048, 2048
SHARD = ROWS // N_CORES  # 256 rows per core
P = 128  # SBUF partitions

last_exec_time_ns = None

_nc_cache: dict[int, bass.Bass] = {}


def _coeffs(T: int) -> tuple[np.ndarray, np.ndarray]:
    lrs = LR0 * GAMMA ** np.arange(T, dtype=np.float64)
    c = np.zeros(T + 1)
    for t in range(T):
        c[t + 1] = (1.0 - lrs[t]) * c[t] + lrs[t]
    a_spike = (0.5 * (c[:T] - 1.0)).astype(np.float32)  # spike_t = a*x + 0.5
    a_y = c[1:].astype(np.float32)  # y_t = a*x
    return a_spike, a_y


def _build(T: int, repeat: int = 1) -> bass.Bass:
    a_spike, a_y = _coeffs(T)
    f32 = mybir.dt.float32

    nc = bacc.Bacc("TRN2", target_bir_lowering=False)
    x = nc.dram_tensor("x", [SHARD, COLS], f32, kind="ExternalInput")
    out = nc.dram_tensor("out", [2, T, SHARD, COLS], f32, kind="ExternalOutput")

    n_row_tiles = SHARD // P  # 2
    with tile.TileContext(nc) as tc:
        with (
            tc.tile_pool(name="xin", bufs=1) as xpool,
            tc.tile_pool(name="obuf", bufs=8) as opool,
        ):
            xts = []
            for i in range(n_row_tiles):
                xt = xpool.tile([P, COLS], f32, tag=f"x{i}")
                nc.sync.dma_start(xt[:], x[i * P : (i + 1) * P, :])
                xts.append(xt)

            def body():
                k = 0
                for t in range(T):
                    for s, a, b in ((0, a_spike[t], 0.5), (1, a_y[t], 0.0)):
                        for i in range(n_row_tiles):
                            ot = opool.tile([P, COLS], f32, tag="o")
                            if k % 2 == 0:
                                nc.vector.tensor_scalar(
                                    ot[:], xts[i][:], float(a), float(b),
                                    mybir.AluOpType.mult, mybir.AluOpType.add,
                                )
                            else:
                                nc.scalar.activation(
                                    ot[:], xts[i][:],
                                    mybir.ActivationFunctionType.Copy,
                                    bias=float(b), scale=float(a),
                                )
                            nc.sync.dma_start(
                                out[s, t, i * P : (i + 1) * P, :], ot[:]
                            )
                            k += 1

            if repeat == 1:
                body()
            else:  # bench-only: amplify HW time so it rises above dispatch floor
                with tc.For_i(0, repeat):
                    body()
    nc.finalize()
    return nc


def kernel(x: np.ndarray, T) -> np.ndarray:
    T = int(T)
    x = np.ascontiguousarray(np.asarray(x), dtype=np.float32)

    if T not in _nc_cache:
        _nc_cache[T] = _build(T)
    nc = _nc_cache[T]

    in_maps = [{"x": x[i * SHARD : (i + 1) * SHARD]} for i in range(N_CORES)]
    res = run_bass_kernel_spmd(nc, in_maps, list(range(N_CORES)))
    return np.concatenate([r["out"] for r in res.results], axis=2)
